# revision 4
# baseline (speedup 1.0000x reference)
"""Trainium2 Bass kernel for nn_CustomPrecision (macro-precision from argmax
confusion matrix) — v3.

Math: t = argmax(y_true, 1), p = argmax(y_pred, 1);
cm = onehot(t)^T @ onehot(p) + confusion_matrix (32x32 histogram);
out = mean(diag(cm) / (colsum(cm) + eps)).

v3 design (8 NeuronCores, data-parallel over N):
  - host packs each core's 250k-row shard (padded to 128x1960 rows with
    argmax-0 pad rows, subtracted from cm[0,0] afterwards) into one DRAM
    tensor laid out chunk-by-chunk: [yt rows | yp rows] per chunk, so
    every chunk is ONE contiguous-per-partition DMA.
  - SWDGE DMA casts f32 -> fp16 in flight (HBM traffic unchanged; DVE
    gets 16-bit 2x mode).  fp16 argmax ties cost ~5e-4 rel err (sim).
  - rowmax via a 5-level TT-max tree on DVE (2x mode, ~0.5 cyc/elem)
    instead of tensor_reduce (1x-capped).
  - rowmax replicated x32 on the otherwise-idle ACT engine (+ Pool for
    a slice) so the DVE is_equal runs with two contiguous fp16 streams
    (2x mode) instead of a step-0 broadcast operand (1x).
  - 32x32 confusion matrix accumulated on the TensorEngine in PSUM:
    one [K=128]x[32x32] fp16 matmul per 128-row group (masks are 0/1 so
    products and fp32 PSUM accumulation are exact).
  - each core DMAs its local cm out; the host sums the 8 tiny matrices,
    subtracts the pad counts, adds confusion_matrix, and does the
    32-element precision reduction (the gather/unshard step).
"""

import numpy as np

import concourse.bass as bass
import concourse.mybir as mybir
import concourse.tile as tile
from concourse.bass_utils import run_bass_kernel_spmd

F32 = mybir.dt.float32
F16 = mybir.dt.float16
BF16 = mybir.dt.bfloat16
AX = mybir.AxisListType
OP = mybir.AluOpType
ACT_ID = mybir.ActivationFunctionType.Identity

NCORES = 8
N = 2_000_000
C = 32
R = N // NCORES              # 250_000 rows per core
P = 128                      # partitions
RPP = 1960                   # padded rows per partition
PAD = P * RPP - R            # 880 pad rows per core (argmax lands in cm[0,0])
EPS = float(np.finfo(np.float32).eps)

# chunk sizes (rows/partition); ramp keeps the pipeline fill short and the
# small trailing chunks shorten the post-last-DMA tail.
CHUNKS = [8, 16, 24, 48, 96] + [128] * 12 + [104, 80, 48]
assert sum(CHUNKS) == RPP

# fraction of the mrep materialization done on ACT (rest on Pool)
ACT_FRAC = 1.0


def _split_multi_waits(nc, max_waits=1):
    """This container's walrus rejects >1 sync-wait per instruction
    ('Too many sync wait commands').  Move excess waits onto same-engine
    InstNoOp's inserted immediately before the carrying instruction —
    the sequencer blocks on each nop first, so gating is preserved."""
    idx = 0
    for bb in nc.main_func.blocks:
        new_list = []
        for ins in bb.instructions:
            si = ins.sync_info
            if si is not None and si.on_wait and len(si.on_wait) > max_waits:
                waits = list(si.on_wait)
                keep = waits[-max_waits:]
                rest = waits[:-max_waits]
                for i in range(0, len(rest), max_waits):
                    nop = mybir.InstNoOp(
                        name=f"splitw-{idx}",
                        engine=ins.engine,
                        ins=[],
                        outs=[],
                        sync_info=mybir.SyncInfo(
                            on_update=[], on_wait=rest[i : i + max_waits]
                        ),
                    )
                    idx += 1
                    nc.register_instruction(nop, overwrite=True)
                    new_list.append(nop)
                ins.sync_info = mybir.SyncInfo(
                    on_update=list(si.on_update or []), on_wait=keep
                )
            new_list.append(ins)
        bb.instructions = new_list


def _build_program():
    nc = bass.Bass("TRN2", num_devices=NCORES)
    xin = nc.dram_tensor("xin", [P, 2 * RPP * C], F32, kind="ExternalInput")
    cm_out = nc.dram_tensor("cm", [C, C], F32, kind="ExternalOutput")

    n_mm = sum(CHUNKS)
    mm_idx = 0

    with tile.TileContext(nc) as tc:
        with (
            tc.tile_pool(name="io", bufs=3) as io_pool,
            tc.tile_pool(name="tree", bufs=3) as tree_pool,
            tc.tile_pool(name="mask", bufs=3) as mask_pool,
            tc.tile_pool(name="fin", bufs=1) as fin_pool,
            tc.tile_pool(name="ps", bufs=1, space="PSUM") as ps_pool,
        ):
            cm_ps = ps_pool.tile([C, C], F32)

            def emit_eq_and_mms(st):
                """eq + matmuls for a staged chunk.  Emitted one chunk late
                so the DVE's in-order queue holds tree_{i+1} BEFORE eq_i —
                the DVE works on the next chunk's tree while ACT builds
                mrep_i, instead of head-of-line blocking on it."""
                nonlocal mm_idx
                x16, mrep, kt = st
                w = 2 * kt
                eq = mask_pool.tile([P, w * C], F16, tag="eq")
                nc.vector.tensor_tensor(eq[:], x16[:], mrep[:], OP.is_equal)
                eq3 = eq[:].rearrange("p (r c) -> p r c", c=C)
                for j in range(kt):
                    nc.tensor.matmul(
                        cm_ps[:],
                        lhsT=eq3[:, j, :],
                        rhs=eq3[:, kt + j, :],
                        start=(mm_idx == 0),
                        stop=(mm_idx == n_mm - 1),
                    )
                    mm_idx += 1

            staged = None
            off = 0
            for ti, kt in enumerate(CHUNKS):
                w = 2 * kt                      # rows in this chunk (yt + yp)
                x16 = io_pool.tile([P, w * C], F16, tag="x")
                nc.gpsimd.dma_start(
                    x16[:], xin.ap()[:, off : off + w * C]
                )
                off += w * C
                x3 = x16[:].rearrange("p (r c) -> p r c", c=C)

                # rowmax: 3 TT-max tree levels (fp16 2x) + 1x reduce over 4
                t16 = tree_pool.tile([P, w * 16], F16, tag="t16")
                t8 = tree_pool.tile([P, w * 8], F16, tag="t8")
                t4 = tree_pool.tile([P, w * 4], F16, tag="t4")
                m = tree_pool.tile([P, w], F16, tag="m")
                t16v = t16[:].rearrange("p (r c) -> p r c", c=16)
                t8v = t8[:].rearrange("p (r c) -> p r c", c=8)
                t4v = t4[:].rearrange("p (r c) -> p r c", c=4)
                nc.vector.tensor_tensor(t16v, x3[:, :, 0:16], x3[:, :, 16:32], OP.max)
                nc.vector.tensor_tensor(t8v, t16v[:, :, 0:8], t16v[:, :, 8:16], OP.max)
                nc.vector.tensor_tensor(t4v, t8v[:, :, 0:4], t8v[:, :, 4:8], OP.max)
                nc.vector.tensor_reduce(m[:], t4v, axis=AX.X, op=OP.max)

                # replicate m across the 32 classes on the ACT engine
                mrep = mask_pool.tile([P, w * C], F16, tag="mrep")
                mrep3 = mrep[:].rearrange("p (r c) -> p r c", c=C)
                mb = m[:].unsqueeze(2).broadcast_to((P, w, C))
                nc.scalar.activation(mrep3, mb, ACT_ID)

                if staged is not None:
                    emit_eq_and_mms(staged)
                staged = (x16, mrep, kt)
            emit_eq_and_mms(staged)

            cm_sb = fin_pool.tile([C, C], F32)
            nc.vector.tensor_copy(cm_sb[:], cm_ps[:])
            nc.sync.dma_start(cm_out.ap()[:, :], cm_sb[:])

    _split_multi_waits(nc)
    return nc


_NC_CACHE = None


def _get_nc():
    global _NC_CACHE
    if _NC_CACHE is None:
        _NC_CACHE = _build_program()
    return _NC_CACHE


def _prep_in_maps(y_true: np.ndarray, y_pred: np.ndarray):
    y_true = np.ascontiguousarray(y_true, dtype=np.float32)
    y_pred = np.ascontiguousarray(y_pred, dtype=np.float32)
    pad_row = np.zeros((C,), np.float32)
    pad_row[0] = 1.0
    # chunk base column offsets: each chunk holds [yt kt*C | yp kt*C]
    bases = []
    o = 0
    for kt in CHUNKS:
        bases.append(o)
        o += 2 * kt * C
    in_maps = []
    for i in range(NCORES):
        xin = np.empty((P, 2 * RPP * C), np.float32)
        for half, src in enumerate((y_true, y_pred)):
            a = np.empty((P * RPP, C), np.float32)
            a[:R] = src[i * R : (i + 1) * R]
            a[R:] = pad_row
            a = a.reshape(P, RPP, C)
            k0 = 0
            for ci, kt in enumerate(CHUNKS):
                o = bases[ci] + half * kt * C
                xin[:, o : o + kt * C] = a[:, k0 : k0 + kt, :].reshape(P, kt * C)
                k0 += kt
        in_maps.append({"xin": xin})
    return in_maps


def _finalize(results, confusion_matrix: np.ndarray) -> np.ndarray:
    cm = np.zeros((C, C), np.float32)
    for r in results:
        cm += r["cm"]
    cm[0, 0] -= np.float32(NCORES * PAD)
    cm = cm + np.asarray(confusion_matrix, dtype=np.float32)
    tp = np.diag(cm)
    pp = cm.sum(axis=0, dtype=np.float32)
    prec = (tp / (pp + np.float32(EPS))).astype(np.float32)
    return np.float32(prec.mean(dtype=np.float32))


def kernel(y_true: np.ndarray, y_pred: np.ndarray,
           confusion_matrix: np.ndarray) -> np.ndarray:
    nc = _get_nc()
    in_maps = _prep_in_maps(y_true, y_pred)
    res = run_bass_kernel_spmd(nc, in_maps, core_ids=list(range(NCORES)))
    return np.asarray(_finalize(res.results, confusion_matrix), dtype=np.float32)


if __name__ == "__main__":
    rng = np.random.default_rng(0)
    yt = rng.standard_normal((N, C), dtype=np.float32)
    yp = rng.standard_normal((N, C), dtype=np.float32)
    cm0 = np.zeros((C, C), np.float32)
    got = kernel(yt, yp, cm0)
    t = yt.argmax(1)
    p = yp.argmax(1)
    cmref = np.zeros((C, C), np.float64)
    np.add.at(cmref, (t, p), 1.0)
    tp = np.diag(cmref)
    ppos = cmref.sum(0)
    want = np.mean((tp / (ppos + EPS)).astype(np.float32))
    print("kernel:", got, "numpy:", want, "relerr:", abs(got - want) / abs(want))
